# revision 15
# baseline (speedup 1.0000x reference)
"""MoE-LoRA linear layer (T=16384, D=1024, E=64, R=8) on 8 Trainium2 cores.

Strategy: data-parallel over tokens (2048 tokens/core). Inside each core
everything is computed transposed (d on partitions, tokens on the free dim)
so every matmul consumes operands in their natural layout with no on-device
transposes:

  out_T[:, g] = sum_k W_k^T @ xT_k[:, g]      base GEMM, N=512 token groups
  out_T[:, b] += B_blk^T @ (mask_b * (A_blk^T @ xT[:, b]))   rank-8 LoRA

Routing is resolved on the host: each core's tokens are sorted by expert
label and cut into 256-token blocks. Because sorted blocks span a small
contiguous expert range, each block reads its LoRA A columns from ONE shared
A-table in SBUF through a compile-time static 16-expert window (elo[b]),
so A is DMA'd once (1 MB) instead of per-block packed copies. B and the
per-token masks are still host-packed per block (slot = e - elo[b]).

The start is latency-optimized: eight small "waves" [x0 chunk k | W chunk k]
(384 KB each) stream in k order, so the first matmul fires after one wave
(~1 MB earlier than a k-pair layout) and the PE never outruns the 16-engine
DMA stream (~420 GB/s aggregate). Group 0 accumulates 6 output j-chunks in
6 PSUM banks during the wave phase (the other 2 banks hold the packed
[128,512] xa pair tiles); j=6,7 run right after from resident tiles. A few
throwaway warm-up matmuls bridge the fixed ~7.5us framework preamble so the
PE clock gate (HAM, 1.2 -> 2.4 GHz) releases early. Output is stored bf16
(halves store traffic; host casts back to f32). Compute in bf16 (f32 PSUM):
fp32 matmul on TRN2 runs at 1/4 rate and would be hopelessly PE-bound.
"""

import numpy as np
import ml_dtypes

import concourse.bacc as bacc
import concourse.mybir as mybir
from concourse import tile
from concourse.bass_utils import run_bass_kernel_spmd

T, D, E, R = 16384, 1024, 64, 8
N_CORES = 8
TPC = T // N_CORES          # tokens per core
KD = D // 128               # 8 contraction chunks
GRP = 512                   # base-GEMM token group (one PSUM bank)
NG = TPC // GRP             # 4 groups
SCALING = 1.0 / R
SLOTS = 128 // R            # experts a 128-row slot window holds
N_WARM = 7                  # HAM warm-up matmuls bridging the DMA lead-in
WVC = GRP + D               # wave row: x0 chunk | W chunk

BF16 = ml_dtypes.bfloat16

_compiled = {}              # n_blocks -> Bacc program (reused across calls)
_last_in_maps = None


def _elo_table(n_blocks: int):
    # static expert window base per block: block b of sorted tokens spans
    # experts around epb*b; center a SLOTS-wide window there.
    epb = E // n_blocks
    return [
        min(max(epb * b - (SLOTS - epb) // 2, 0), E - SLOTS)
        for b in range(n_blocks)
    ]


def _build_nc(n_blocks: int):
    blk = TPC // n_blocks   # lora block (256 default)
    sub = GRP // blk        # lora blocks per token group
    elo = _elo_table(n_blocks)
    bf = mybir.dt.bfloat16
    f32 = mybir.dt.float32

    nc = bacc.Bacc(
        "TRN2", target_bir_lowering=False, debug=False, num_devices=N_CORES
    )
    wv_d = nc.dram_tensor("wv", [KD, 128, WVC], bf, kind="ExternalInput")
    a_d = nc.dram_tensor("atab", [128, KD * E * R], bf, kind="ExternalInput")
    lt_d = [
        nc.dram_tensor(f"lt{g}", [128, sub * D + GRP], bf, kind="ExternalInput")
        for g in range(NG)
    ]
    xr_d = nc.dram_tensor("xr", [NG - 1, 128, KD * GRP], bf, kind="ExternalInput")
    bias_d = nc.dram_tensor("bias", [128, KD], f32, kind="ExternalInput")
    out_d = nc.dram_tensor("outT", [KD, 128, TPC], bf, kind="ExternalOutput")

    with tile.TileContext(nc) as tc:
        with (
            tc.tile_pool(name="consts", bufs=1) as cpool,
            tc.tile_pool(name="xa_ps", bufs=1, space="PSUM") as xa_ps,
            tc.tile_pool(name="out_ps", bufs=7, space="PSUM") as out_ps,
            tc.tile_pool(name="xm_sb", bufs=2) as xm_pool,
            tc.tile_pool(name="st_sb", bufs=12) as st_pool,
        ):
            wv_t = [
                cpool.tile([128, WVC], bf, tag=f"wv{k}", name=f"wv_t{k}")
                for k in range(KD)
            ]
            a_t = cpool.tile([128, KD * E * R], bf, tag="atab", name="a_t")
            lt_t = [
                cpool.tile([128, sub * D + GRP], bf, tag=f"lt{g}", name=f"lt_t{g}")
                for g in range(NG)
            ]
            xr_t = [
                cpool.tile([128, KD * GRP], bf, tag=f"xr{g}", name=f"xr_t{g}")
                for g in range(NG - 1)
            ]
            bias_t = cpool.tile([128, KD], f32, tag="bias", name="bias_t")
            warm_sb = cpool.tile([128, GRP], bf, tag="warm", name="warm_sb")

            # waves alternate between the two HWDGE rings (sync / ACT): the
            # ~0.6us per-DMA descriptor-gen otherwise serializes arrivals
            # (0.91us stream + 0.6us issue > the PE's 1.49us/wave pace)
            for k in range(KD):
                eng = nc.sync if k % 2 == 0 else nc.scalar
                eng.dma_start(wv_t[k][:], wv_d[k, :, :])
            # remaining inputs ride the ACT ring BEHIND the odd waves, so
            # they cannot steal engine bandwidth from the wave stream; the
            # sync ring stays clear for the output stores
            nc.sync.dma_start(bias_t[:], bias_d[:, :])
            nc.scalar.dma_start(a_t[:], a_d[:, :])
            nc.scalar.dma_start(lt_t[0][:], lt_d[0][:, :])
            nc.scalar.dma_start(xr_t[0][:], xr_d[0, :, :])
            nc.scalar.dma_start(lt_t[1][:], lt_d[1][:, :])
            nc.scalar.dma_start(xr_t[1][:], xr_d[1, :, :])
            nc.scalar.dma_start(lt_t[2][:], lt_d[2][:, :])
            nc.scalar.dma_start(xr_t[2][:], xr_d[2, :, :])
            nc.scalar.dma_start(lt_t[3][:], lt_d[3][:, :])

            def w_sl(k, j):
                return wv_t[k][:, GRP + j * 128 : GRP + (j + 1) * 128]

            def x_sl(g, k, c0, c1):
                if g == 0:
                    return wv_t[k][:, c0:c1]
                return xr_t[g - 1][:, k * GRP + c0 : k * GRP + c1]

            def a_sl(b, k):
                o = k * E * R + elo[b] * R
                return a_t[:, o : o + 128]

            def b_sl(g, h, j):
                return lt_t[g][:, h * D + j * 128 : h * D + (j + 1) * 128]

            def m_sl(g):
                return lt_t[g][:, sub * D : sub * D + GRP]

            # PE warm-up across the fixed framework preamble / first-wave DMA
            nc.vector.memset(warm_sb[:], 0.0)
            for i in range(N_WARM):
                wp = out_ps.tile([128, GRP], f32, tag="o", name=f"warm_ps{i}")
                nc.tensor.matmul(
                    wp[:],
                    lhsT=warm_sb[:, 0:128],
                    rhs=warm_sb[:],
                    start=True,
                    stop=True,
                    skip_group_check=True,
                )

            xm = [None] * NG

            def emit_xa(g):
                # xa[slot, t] for both lora blocks of group g, packed into
                # one PSUM bank; each block's first write lands on cleared
                # has_written bits so start=True is only needed once.
                xa = xa_ps.tile([128, GRP], f32, tag="xa", name=f"xa{g}")
                for h in range(sub):
                    b = g * sub + h
                    for k in range(KD):
                        nc.tensor.matmul(
                            xa[:, h * blk : (h + 1) * blk],
                            lhsT=a_sl(b, k),
                            rhs=x_sl(g, k, h * blk, (h + 1) * blk),
                            start=(h == 0 and k == 0),
                            stop=(h == sub - 1 and k == KD - 1),
                            skip_group_check=True,
                        )
                t = xm_pool.tile([128, GRP], bf, tag="xm", name=f"xm{g}")
                nc.vector.tensor_mul(t[:], xa[:], m_sl(g))
                xm[g] = t

            def emit_base(g, j, o_p, ks):
                for k in ks:
                    nc.tensor.matmul(
                        o_p[:],
                        lhsT=w_sl(k, j),
                        rhs=x_sl(g, k, 0, GRP),
                        start=(k == 0),
                        stop=False,
                        skip_group_check=True,
                    )

            def finish(g, j, o_p):
                for h in range(sub):
                    nc.tensor.matmul(
                        o_p[:, h * blk : (h + 1) * blk],
                        lhsT=b_sl(g, h, j),
                        rhs=xm[g][:, h * blk : (h + 1) * blk],
                        start=False,
                        stop=(h == sub - 1),
                        skip_group_check=True,
                    )
                st = st_pool.tile([128, GRP], bf, tag="st", name=f"st{g}_{j}")
                # bias-add on the ACT engine: keeps the DVE free for the mask
                # muls so no cross-engine wait ever blocks the DVE FIFO head
                nc.scalar.add(st[:], o_p[:], bias_t[:, j : j + 1])
                # stores issue on sync: ACT-issued DMAs wait on sem-lane
                # recycling gates and would block later adds in the ACT FIFO
                nc.sync.dma_start(out_d[j, :, g * GRP : (g + 1) * GRP], st[:])

            # --- group 0: wave phase, 7 persistent banks in k order ---
            o7 = [
                out_ps.tile([128, GRP], f32, tag="o", name=f"o0_{j}")
                for j in range(7)
            ]
            for k in range(KD):
                for j in range(7):
                    nc.tensor.matmul(
                        o7[j][:],
                        lhsT=w_sl(k, j),
                        rhs=wv_t[k][:, 0:GRP],
                        start=(k == 0),
                        stop=False,
                        skip_group_check=True,
                    )
            emit_xa(0)
            for j in range(7):
                finish(0, j, o7[j])
                if j == 2:
                    emit_xa(1)
            o_p = out_ps.tile([128, GRP], f32, tag="o", name="o0_7")
            emit_base(0, 7, o_p, range(KD))
            finish(0, 7, o_p)

            # --- groups 1..3: straight pipeline; final j split in half so
            # the last DVE+store chain overlaps the closing matmuls ---
            for g in range(1, NG):
                for j in range(KD):
                    last = g == NG - 1 and j == KD - 1
                    if not last:
                        o_p = out_ps.tile(
                            [128, GRP], f32, tag="o", name=f"o{g}_{j}"
                        )
                        emit_base(g, j, o_p, range(KD))
                        finish(g, j, o_p)
                        if j == 3 and g < NG - 1:
                            emit_xa(g + 1)
                        continue
                    # split halves: each half gets its own PSUM bank so the
                    # first half's bias+store runs while the second half's
                    # matmuls still stream (PE-W + DVE-R same bank is fatal)
                    for h in range(sub):
                        o_h = out_ps.tile(
                            [128, GRP], f32, tag="o", name=f"o{g}_{j}_h{h}"
                        )
                        c0, c1 = h * blk, (h + 1) * blk
                        for k in range(KD):
                            nc.tensor.matmul(
                                o_h[:, 0:blk],
                                lhsT=w_sl(k, j),
                                rhs=x_sl(g, k, c0, c1),
                                start=(k == 0),
                                stop=False,
                                skip_group_check=True,
                            )
                        nc.tensor.matmul(
                            o_h[:, 0:blk],
                            lhsT=b_sl(g, h, j),
                            rhs=xm[g][:, c0:c1],
                            start=False,
                            stop=True,
                            skip_group_check=True,
                        )
                        st = st_pool.tile(
                            [128, blk], bf, tag="sth", name=f"st{g}_{j}_h{h}"
                        )
                        nc.scalar.add(st[:], o_h[:, 0:blk], bias_t[:, j : j + 1])
                        nc.sync.dma_start(
                            out_d[j, :, g * GRP + c0 : g * GRP + c1], st[:]
                        )

    nc.compile()
    return nc


def _pick_blocks(labels: np.ndarray):
    # smallest block count whose static expert windows cover every core's
    # sorted blocks (sorted blocks span a contiguous expert range)
    for n_blocks in (8, 16, 32):
        blk = TPC // n_blocks
        elo = _elo_table(n_blocks)
        ok = True
        for c in range(N_CORES):
            sl = np.sort(labels[c * TPC : (c + 1) * TPC])
            for b in range(n_blocks):
                seg = sl[b * blk : (b + 1) * blk]
                if seg[0] < elo[b] or seg[-1] >= elo[b] + SLOTS:
                    ok = False
                    break
            if not ok:
                break
        if ok:
            return n_blocks, elo
    raise ValueError("no block size with a static 16-expert window fits")


def kernel(x, labels, W, A, B, bias):
    global _last_in_maps
    x = np.asarray(x, dtype=np.float32)
    labels_i = np.asarray(labels).astype(np.int64)
    W = np.asarray(W, dtype=np.float32)
    A = np.asarray(A, dtype=np.float32)
    B = np.asarray(B, dtype=np.float32)
    bias = np.asarray(bias, dtype=np.float32)

    n_blocks, elo = _pick_blocks(labels_i)
    blk = TPC // n_blocks
    sub = GRP // blk

    if n_blocks not in _compiled:
        _compiled[n_blocks] = _build_nc(n_blocks)
    nc = _compiled[n_blocks]

    w_part = W.reshape(KD, 128, D).astype(BF16)             # [k, p, j*128+..]
    bias_in = np.ascontiguousarray(bias.reshape(KD, 128).T)  # [128, KD] f32
    B_scaled = B * SCALING
    # shared A table: atab[p, k*E*R + e*R + r] = A[e, 128k+p, r]
    a_tab = np.ascontiguousarray(
        A.astype(BF16).reshape(E, KD, 128, R).transpose(2, 1, 0, 3).reshape(
            128, KD * E * R
        )
    )

    in_maps = []
    perms = []
    for c in range(N_CORES):
        lc = labels_i[c * TPC : (c + 1) * TPC]
        perm = np.argsort(lc, kind="stable")
        perms.append(perm)
        ls = lc[perm]                          # sorted labels
        xs = x[c * TPC : (c + 1) * TPC][perm]  # [TPC, D] sorted tokens

        # xt[k, p, g, t] = xs[g*GRP + t, 128k + p]
        xt = xs.astype(BF16).T.reshape(KD, 128, NG, GRP)
        wv_in = np.ascontiguousarray(
            np.concatenate([xt[:, :, 0, :], w_part], axis=2)  # [KD, 128, WVC]
        )
        xr_in = np.ascontiguousarray(
            xt[:, :, 1:, :].transpose(2, 1, 0, 3).reshape(NG - 1, 128, KD * GRP)
        )

        in_map = {"wv": wv_in, "atab": a_tab, "bias": bias_in, "xr": xr_in}
        for g in range(NG):
            ltg = np.zeros((128, sub * D + GRP), dtype=BF16)
            for h in range(sub):
                b = g * sub + h
                seg = ls[b * blk : (b + 1) * blk]
                lo = elo[b]
                for e in np.unique(seg):
                    i = int(e) - lo
                    assert 0 <= i < SLOTS
                    ltg[i * R : (i + 1) * R, h * D : (h + 1) * D] = B_scaled[e]
                    ltg[
                        i * R : (i + 1) * R,
                        sub * D + h * blk : sub * D + (h + 1) * blk,
                    ] = (seg == e)[None, :]
            in_map[f"lt{g}"] = ltg
        in_maps.append(in_map)

    _last_in_maps = in_maps
    res = run_bass_kernel_spmd(nc, in_maps, core_ids=list(range(N_CORES)))

    out = np.empty((T, D), dtype=np.float32)
    for c in range(N_CORES):
        o_t = res.results[c]["outT"].reshape(D, TPC)  # [d, t] sorted, bf16
        out[c * TPC + perms[c]] = o_t.T.astype(np.float32)
    return out


# revision 16
# speedup vs baseline: 1.0309x; 1.0309x over previous
"""MoE-LoRA linear layer (T=16384, D=1024, E=64, R=8) on 8 Trainium2 cores.

Strategy: data-parallel over tokens (2048 tokens/core). Inside each core
everything is computed transposed (d on partitions, tokens on the free dim)
so every matmul consumes operands in their natural layout with no on-device
transposes:

  out_T[:, g] = sum_k W_k^T @ xT_k[:, g]      base GEMM, N=512 token groups
  out_T[:, b] += B_blk^T @ (mask_b * (A_blk^T @ xT[:, b]))   rank-8 LoRA

Routing is resolved on the host: each core's tokens are sorted by expert
label and cut into 256-token blocks. Because sorted blocks span a small
contiguous expert range, each block reads its LoRA A columns from ONE shared
A-table in SBUF through a compile-time static 16-expert window (elo[b]),
so A is DMA'd once (1 MB) instead of per-block packed copies. B and the
per-token masks are still host-packed per block (slot = e - elo[b]).

The start is latency-optimized: eight small "waves" [x0 chunk k | W chunk k]
(384 KB each) stream in k order, so the first matmul fires after one wave
(~1 MB earlier than a k-pair layout) and the PE never outruns the 16-engine
DMA stream (~420 GB/s aggregate). Group 0 accumulates 6 output j-chunks in
6 PSUM banks during the wave phase (the other 2 banks hold the packed
[128,512] xa pair tiles); j=6,7 run right after from resident tiles. A few
throwaway warm-up matmuls bridge the fixed ~7.5us framework preamble so the
PE clock gate (HAM, 1.2 -> 2.4 GHz) releases early. Output is stored bf16
(halves store traffic; host casts back to f32). Compute in bf16 (f32 PSUM):
fp32 matmul on TRN2 runs at 1/4 rate and would be hopelessly PE-bound.
"""

import numpy as np
import ml_dtypes

import concourse.bacc as bacc
import concourse.mybir as mybir
from concourse import tile
from concourse.bass_utils import run_bass_kernel_spmd

T, D, E, R = 16384, 1024, 64, 8
N_CORES = 8
TPC = T // N_CORES          # tokens per core
KD = D // 128               # 8 contraction chunks
GRP = 512                   # base-GEMM token group (one PSUM bank)
NG = TPC // GRP             # 4 groups
SCALING = 1.0 / R
SLOTS = 128 // R            # experts a 128-row slot window holds
N_WARM = 7                  # HAM warm-up matmuls bridging the DMA lead-in
WVC = GRP + D               # wave row: x0 chunk | W chunk

BF16 = ml_dtypes.bfloat16

_compiled = {}              # n_blocks -> Bacc program (reused across calls)
_last_in_maps = None


def _elo_table(n_blocks: int):
    # static expert window base per block: block b of sorted tokens spans
    # experts around epb*b; center a SLOTS-wide window there.
    epb = E // n_blocks
    return [
        min(max(epb * b - (SLOTS - epb) // 2, 0), E - SLOTS)
        for b in range(n_blocks)
    ]


def _build_nc(n_blocks: int):
    blk = TPC // n_blocks   # lora block (256 default)
    sub = GRP // blk        # lora blocks per token group
    elo = _elo_table(n_blocks)
    bf = mybir.dt.bfloat16
    f32 = mybir.dt.float32

    nc = bacc.Bacc(
        "TRN2", target_bir_lowering=False, debug=False, num_devices=N_CORES
    )
    wv_d = nc.dram_tensor("wv", [KD, 128, WVC], bf, kind="ExternalInput")
    a_d = nc.dram_tensor("atab", [128, KD * E * R], bf, kind="ExternalInput")
    lt_d = [
        nc.dram_tensor(f"lt{g}", [128, sub * D + GRP], bf, kind="ExternalInput")
        for g in range(NG)
    ]
    xr_d = nc.dram_tensor("xr", [NG - 1, 128, KD * GRP], bf, kind="ExternalInput")
    bias_d = nc.dram_tensor("bias", [128, KD], f32, kind="ExternalInput")
    out_d = nc.dram_tensor("outT", [KD, 128, TPC], bf, kind="ExternalOutput")

    with tile.TileContext(nc) as tc:
        with (
            tc.tile_pool(name="consts", bufs=1) as cpool,
            tc.tile_pool(name="xa_ps", bufs=1, space="PSUM") as xa_ps,
            tc.tile_pool(name="out_ps", bufs=7, space="PSUM") as out_ps,
            tc.tile_pool(name="xm_sb", bufs=2) as xm_pool,
            tc.tile_pool(name="st_sb", bufs=12) as st_pool,
        ):
            wv_t = [
                cpool.tile([128, WVC], bf, tag=f"wv{k}", name=f"wv_t{k}")
                for k in range(KD)
            ]
            a_t = cpool.tile([128, KD * E * R], bf, tag="atab", name="a_t")
            lt_t = [
                cpool.tile([128, sub * D + GRP], bf, tag=f"lt{g}", name=f"lt_t{g}")
                for g in range(NG)
            ]
            xr_t = [
                cpool.tile([128, KD * GRP], bf, tag=f"xr{g}", name=f"xr_t{g}")
                for g in range(NG - 1)
            ]
            bias_t = cpool.tile([128, KD], f32, tag="bias", name="bias_t")
            warm_sb = cpool.tile([128, GRP], bf, tag="warm", name="warm_sb")

            # issue order == arrival order on the sync HWDGE ring; every
            # tensor is ordered by first use (ACT-ring experiments measured
            # neutral-to-worse: sem-lane gates + ring competition)
            for k in range(KD):
                nc.sync.dma_start(wv_t[k][:], wv_d[k, :, :])
            nc.sync.dma_start(a_t[:], a_d[:, :])
            nc.sync.dma_start(lt_t[0][:], lt_d[0][:, :])
            nc.sync.dma_start(bias_t[:], bias_d[:, :])
            nc.sync.dma_start(xr_t[0][:], xr_d[0, :, :])
            nc.sync.dma_start(lt_t[1][:], lt_d[1][:, :])
            nc.sync.dma_start(xr_t[1][:], xr_d[1, :, :])
            nc.sync.dma_start(lt_t[2][:], lt_d[2][:, :])
            nc.sync.dma_start(xr_t[2][:], xr_d[2, :, :])
            nc.sync.dma_start(lt_t[3][:], lt_d[3][:, :])

            def w_sl(k, j):
                return wv_t[k][:, GRP + j * 128 : GRP + (j + 1) * 128]

            def x_sl(g, k, c0, c1):
                if g == 0:
                    return wv_t[k][:, c0:c1]
                return xr_t[g - 1][:, k * GRP + c0 : k * GRP + c1]

            def a_sl(b, k):
                o = k * E * R + elo[b] * R
                return a_t[:, o : o + 128]

            def b_sl(g, h, j):
                return lt_t[g][:, h * D + j * 128 : h * D + (j + 1) * 128]

            def m_sl(g):
                return lt_t[g][:, sub * D : sub * D + GRP]

            # PE warm-up across the fixed framework preamble / first-wave DMA
            nc.vector.memset(warm_sb[:], 0.0)
            for i in range(N_WARM):
                wp = out_ps.tile([128, GRP], f32, tag="o", name=f"warm_ps{i}")
                nc.tensor.matmul(
                    wp[:],
                    lhsT=warm_sb[:, 0:128],
                    rhs=warm_sb[:],
                    start=True,
                    stop=True,
                    skip_group_check=True,
                )

            xm = [None] * NG

            def emit_xa(g):
                # xa[slot, t] for both lora blocks of group g, packed into
                # one PSUM bank; each block's first write lands on cleared
                # has_written bits so start=True is only needed once.
                xa = xa_ps.tile([128, GRP], f32, tag="xa", name=f"xa{g}")
                for h in range(sub):
                    b = g * sub + h
                    for k in range(KD):
                        nc.tensor.matmul(
                            xa[:, h * blk : (h + 1) * blk],
                            lhsT=a_sl(b, k),
                            rhs=x_sl(g, k, h * blk, (h + 1) * blk),
                            start=(h == 0 and k == 0),
                            stop=(h == sub - 1 and k == KD - 1),
                            skip_group_check=True,
                        )
                t = xm_pool.tile([128, GRP], bf, tag="xm", name=f"xm{g}")
                nc.vector.tensor_mul(t[:], xa[:], m_sl(g))
                xm[g] = t

            def emit_base(g, j, o_p, ks):
                for k in ks:
                    nc.tensor.matmul(
                        o_p[:],
                        lhsT=w_sl(k, j),
                        rhs=x_sl(g, k, 0, GRP),
                        start=(k == 0),
                        stop=False,
                        skip_group_check=True,
                    )

            def finish(g, j, o_p):
                for h in range(sub):
                    nc.tensor.matmul(
                        o_p[:, h * blk : (h + 1) * blk],
                        lhsT=b_sl(g, h, j),
                        rhs=xm[g][:, h * blk : (h + 1) * blk],
                        start=False,
                        stop=(h == sub - 1),
                        skip_group_check=True,
                    )
                st = st_pool.tile([128, GRP], bf, tag="st", name=f"st{g}_{j}")
                # bias-add on the ACT engine: keeps the DVE free for the mask
                # muls so no cross-engine wait ever blocks the DVE FIFO head
                nc.scalar.add(st[:], o_p[:], bias_t[:, j : j + 1])
                # stores issue on sync: ACT-issued DMAs wait on sem-lane
                # recycling gates and would block later adds in the ACT FIFO
                nc.sync.dma_start(out_d[j, :, g * GRP : (g + 1) * GRP], st[:])

            # --- group 0: wave phase, 7 persistent banks in k order ---
            o7 = [
                out_ps.tile([128, GRP], f32, tag="o", name=f"o0_{j}")
                for j in range(7)
            ]
            for k in range(KD):
                for j in range(7):
                    nc.tensor.matmul(
                        o7[j][:],
                        lhsT=w_sl(k, j),
                        rhs=wv_t[k][:, 0:GRP],
                        start=(k == 0),
                        stop=False,
                        skip_group_check=True,
                    )
            emit_xa(0)
            for j in range(7):
                finish(0, j, o7[j])
                if j == 2:
                    emit_xa(1)
            o_p = out_ps.tile([128, GRP], f32, tag="o", name="o0_7")
            emit_base(0, 7, o_p, range(KD))
            finish(0, 7, o_p)

            # --- groups 1..3: straight pipeline; final j split in half so
            # the last DVE+store chain overlaps the closing matmuls ---
            for g in range(1, NG):
                for j in range(KD):
                    last = g == NG - 1 and j == KD - 1
                    if not last:
                        o_p = out_ps.tile(
                            [128, GRP], f32, tag="o", name=f"o{g}_{j}"
                        )
                        emit_base(g, j, o_p, range(KD))
                        finish(g, j, o_p)
                        if j == 3 and g < NG - 1:
                            emit_xa(g + 1)
                        continue
                    # split halves: each half gets its own PSUM bank so the
                    # first half's bias+store runs while the second half's
                    # matmuls still stream (PE-W + DVE-R same bank is fatal)
                    for h in range(sub):
                        o_h = out_ps.tile(
                            [128, GRP], f32, tag="o", name=f"o{g}_{j}_h{h}"
                        )
                        c0, c1 = h * blk, (h + 1) * blk
                        for k in range(KD):
                            nc.tensor.matmul(
                                o_h[:, 0:blk],
                                lhsT=w_sl(k, j),
                                rhs=x_sl(g, k, c0, c1),
                                start=(k == 0),
                                stop=False,
                                skip_group_check=True,
                            )
                        nc.tensor.matmul(
                            o_h[:, 0:blk],
                            lhsT=b_sl(g, h, j),
                            rhs=xm[g][:, c0:c1],
                            start=False,
                            stop=True,
                            skip_group_check=True,
                        )
                        st = st_pool.tile(
                            [128, blk], bf, tag="sth", name=f"st{g}_{j}_h{h}"
                        )
                        nc.scalar.add(st[:], o_h[:, 0:blk], bias_t[:, j : j + 1])
                        nc.sync.dma_start(
                            out_d[j, :, g * GRP + c0 : g * GRP + c1], st[:]
                        )

    nc.compile()
    return nc


def _pick_blocks(labels: np.ndarray):
    # smallest block count whose static expert windows cover every core's
    # sorted blocks (sorted blocks span a contiguous expert range)
    for n_blocks in (8, 16, 32):
        blk = TPC // n_blocks
        elo = _elo_table(n_blocks)
        ok = True
        for c in range(N_CORES):
            sl = np.sort(labels[c * TPC : (c + 1) * TPC])
            for b in range(n_blocks):
                seg = sl[b * blk : (b + 1) * blk]
                if seg[0] < elo[b] or seg[-1] >= elo[b] + SLOTS:
                    ok = False
                    break
            if not ok:
                break
        if ok:
            return n_blocks, elo
    raise ValueError("no block size with a static 16-expert window fits")


def kernel(x, labels, W, A, B, bias):
    global _last_in_maps
    x = np.asarray(x, dtype=np.float32)
    labels_i = np.asarray(labels).astype(np.int64)
    W = np.asarray(W, dtype=np.float32)
    A = np.asarray(A, dtype=np.float32)
    B = np.asarray(B, dtype=np.float32)
    bias = np.asarray(bias, dtype=np.float32)

    n_blocks, elo = _pick_blocks(labels_i)
    blk = TPC // n_blocks
    sub = GRP // blk

    if n_blocks not in _compiled:
        _compiled[n_blocks] = _build_nc(n_blocks)
    nc = _compiled[n_blocks]

    w_part = W.reshape(KD, 128, D).astype(BF16)             # [k, p, j*128+..]
    bias_in = np.ascontiguousarray(bias.reshape(KD, 128).T)  # [128, KD] f32
    B_scaled = B * SCALING
    # shared A table: atab[p, k*E*R + e*R + r] = A[e, 128k+p, r]
    a_tab = np.ascontiguousarray(
        A.astype(BF16).reshape(E, KD, 128, R).transpose(2, 1, 0, 3).reshape(
            128, KD * E * R
        )
    )

    in_maps = []
    perms = []
    for c in range(N_CORES):
        lc = labels_i[c * TPC : (c + 1) * TPC]
        perm = np.argsort(lc, kind="stable")
        perms.append(perm)
        ls = lc[perm]                          # sorted labels
        xs = x[c * TPC : (c + 1) * TPC][perm]  # [TPC, D] sorted tokens

        # xt[k, p, g, t] = xs[g*GRP + t, 128k + p]
        xt = xs.astype(BF16).T.reshape(KD, 128, NG, GRP)
        wv_in = np.ascontiguousarray(
            np.concatenate([xt[:, :, 0, :], w_part], axis=2)  # [KD, 128, WVC]
        )
        xr_in = np.ascontiguousarray(
            xt[:, :, 1:, :].transpose(2, 1, 0, 3).reshape(NG - 1, 128, KD * GRP)
        )

        in_map = {"wv": wv_in, "atab": a_tab, "bias": bias_in, "xr": xr_in}
        for g in range(NG):
            ltg = np.zeros((128, sub * D + GRP), dtype=BF16)
            for h in range(sub):
                b = g * sub + h
                seg = ls[b * blk : (b + 1) * blk]
                lo = elo[b]
                for e in np.unique(seg):
                    i = int(e) - lo
                    assert 0 <= i < SLOTS
                    ltg[i * R : (i + 1) * R, h * D : (h + 1) * D] = B_scaled[e]
                    ltg[
                        i * R : (i + 1) * R,
                        sub * D + h * blk : sub * D + (h + 1) * blk,
                    ] = (seg == e)[None, :]
            in_map[f"lt{g}"] = ltg
        in_maps.append(in_map)

    _last_in_maps = in_maps
    res = run_bass_kernel_spmd(nc, in_maps, core_ids=list(range(N_CORES)))

    out = np.empty((T, D), dtype=np.float32)
    for c in range(N_CORES):
        o_t = res.results[c]["outT"].reshape(D, TPC)  # [d, t] sorted, bf16
        out[c * TPC + perms[c]] = o_t.T.astype(np.float32)
    return out


# revision 18
# speedup vs baseline: 1.0508x; 1.0192x over previous
"""MoE-LoRA linear layer (T=16384, D=1024, E=64, R=8) on 8 Trainium2 cores.

Strategy: data-parallel over tokens (2048 tokens/core). Inside each core
everything is computed transposed (d on partitions, tokens on the free dim)
so every matmul consumes operands in their natural layout with no on-device
transposes:

  out_T[:, g] = sum_k W_k^T @ xT_k[:, g]      base GEMM, N=512 token groups
  out_T[:, b] += B_blk^T @ (mask_b * (A_blk^T @ xT[:, b]))   rank-8 LoRA

Routing is resolved on the host: each core's tokens are sorted by expert
label and cut into 256-token blocks. Because sorted blocks span a small
contiguous expert range, each block reads its LoRA A columns from ONE shared
A-table in SBUF through a compile-time static 16-expert window (elo[b]),
so A is DMA'd once (1 MB) instead of per-block packed copies. B and the
per-token masks are still host-packed per block (slot = e - elo[b]).

The start is latency-optimized: eight small "waves" [x0 chunk k | W chunk k]
(384 KB each) stream in k order, so the first matmul fires after one wave
(~1 MB earlier than a k-pair layout) and the PE never outruns the 16-engine
DMA stream (~420 GB/s aggregate). Group 0 accumulates 6 output j-chunks in
6 PSUM banks during the wave phase (the other 2 banks hold the packed
[128,512] xa pair tiles); j=6,7 run right after from resident tiles. A few
throwaway warm-up matmuls bridge the fixed ~7.5us framework preamble so the
PE clock gate (HAM, 1.2 -> 2.4 GHz) releases early. Output is stored bf16
(halves store traffic; host casts back to f32). Compute in bf16 (f32 PSUM):
fp32 matmul on TRN2 runs at 1/4 rate and would be hopelessly PE-bound.
"""

import numpy as np
import ml_dtypes

import concourse.bacc as bacc
import concourse.mybir as mybir
from concourse import tile
from concourse.bass_utils import run_bass_kernel_spmd

T, D, E, R = 16384, 1024, 64, 8
N_CORES = 8
TPC = T // N_CORES          # tokens per core
KD = D // 128               # 8 contraction chunks
GRP = 512                   # base-GEMM token group (one PSUM bank)
NG = TPC // GRP             # 4 groups
SCALING = 1.0 / R
SLOTS = 128 // R            # experts a 128-row slot window holds
N_WARM = 7                  # HAM warm-up matmuls bridging the DMA lead-in
WVC = GRP + D               # wave row: x0 chunk | W chunk

BF16 = ml_dtypes.bfloat16

_compiled = {}              # n_blocks -> Bacc program (reused across calls)
_last_in_maps = None


def _elo_table(n_blocks: int):
    # static expert window base per block: block b of sorted tokens spans
    # experts around epb*b; center a SLOTS-wide window there.
    epb = E // n_blocks
    return [
        min(max(epb * b - (SLOTS - epb) // 2, 0), E - SLOTS)
        for b in range(n_blocks)
    ]


def _build_nc(n_blocks: int):
    blk = TPC // n_blocks   # lora block (256 default)
    sub = GRP // blk        # lora blocks per token group
    elo = _elo_table(n_blocks)
    bf = mybir.dt.bfloat16
    f32 = mybir.dt.float32

    nc = bacc.Bacc(
        "TRN2", target_bir_lowering=False, debug=False, num_devices=N_CORES
    )
    wv_d = nc.dram_tensor("wv", [KD, 128, WVC], bf, kind="ExternalInput")
    a_d = nc.dram_tensor("atab", [128, KD * E * R], bf, kind="ExternalInput")
    lt_d = [
        nc.dram_tensor(f"lt{g}", [128, sub * D + GRP], bf, kind="ExternalInput")
        for g in range(NG)
    ]
    xr_d = nc.dram_tensor("xr", [NG - 1, 128, KD * GRP], bf, kind="ExternalInput")
    bias_d = nc.dram_tensor("bias", [128, KD], f32, kind="ExternalInput")
    out_d = nc.dram_tensor("outT", [KD, 128, TPC], bf, kind="ExternalOutput")

    with tile.TileContext(nc) as tc:
        with (
            tc.tile_pool(name="consts", bufs=1) as cpool,
            tc.tile_pool(name="xa_ps", bufs=1, space="PSUM") as xa_ps,
            tc.tile_pool(name="out_ps", bufs=7, space="PSUM") as out_ps,
            tc.tile_pool(name="xm_sb", bufs=2) as xm_pool,
            tc.tile_pool(name="st_sb", bufs=12) as st_pool,
        ):
            wv_t = [
                cpool.tile([128, WVC], bf, tag=f"wv{k}", name=f"wv_t{k}")
                for k in range(KD)
            ]
            a_t = cpool.tile([128, KD * E * R], bf, tag="atab", name="a_t")
            lt_t = [
                cpool.tile([128, sub * D + GRP], bf, tag=f"lt{g}", name=f"lt_t{g}")
                for g in range(NG)
            ]
            xr_t = [
                cpool.tile([128, KD * GRP], bf, tag=f"xr{g}", name=f"xr_t{g}")
                for g in range(NG - 1)
            ]
            bias_t = cpool.tile([128, KD], f32, tag="bias", name="bias_t")
            warm_sb = cpool.tile([128, GRP], bf, tag="warm", name="warm_sb")

            # issue order == arrival order on the sync HWDGE ring; every
            # tensor is ordered by first use (ACT-ring experiments measured
            # neutral-to-worse: sem-lane gates + ring competition)
            for k in range(KD):
                nc.sync.dma_start(wv_t[k][:], wv_d[k, :, :])
            nc.sync.dma_start(a_t[:], a_d[:, :])
            nc.sync.dma_start(lt_t[0][:], lt_d[0][:, :])
            nc.sync.dma_start(bias_t[:], bias_d[:, :])
            nc.sync.dma_start(xr_t[0][:], xr_d[0, :, :])
            nc.sync.dma_start(lt_t[1][:], lt_d[1][:, :])
            nc.sync.dma_start(xr_t[1][:], xr_d[1, :, :])
            nc.sync.dma_start(lt_t[2][:], lt_d[2][:, :])
            nc.sync.dma_start(xr_t[2][:], xr_d[2, :, :])
            nc.sync.dma_start(lt_t[3][:], lt_d[3][:, :])

            def w_sl(k, j):
                return wv_t[k][:, GRP + j * 128 : GRP + (j + 1) * 128]

            def x_sl(g, k, c0, c1):
                if g == 0:
                    return wv_t[k][:, c0:c1]
                return xr_t[g - 1][:, k * GRP + c0 : k * GRP + c1]

            def a_sl(b, k):
                o = k * E * R + elo[b] * R
                return a_t[:, o : o + 128]

            def b_sl(g, h, j):
                return lt_t[g][:, h * D + j * 128 : h * D + (j + 1) * 128]

            def m_sl(g):
                return lt_t[g][:, sub * D : sub * D + GRP]

            # PE warm-up across the fixed framework preamble / first-wave DMA
            nc.vector.memset(warm_sb[:], 0.0)
            for i in range(N_WARM):
                wp = out_ps.tile([128, GRP], f32, tag="o", name=f"warm_ps{i}")
                nc.tensor.matmul(
                    wp[:],
                    lhsT=warm_sb[:, 0:128],
                    rhs=warm_sb[:],
                    start=True,
                    stop=True,
                    skip_group_check=True,
                )

            xm = [None] * NG

            def emit_xa(g):
                # xa[slot, t] for both lora blocks of group g, packed into
                # one PSUM bank; each block's first write lands on cleared
                # has_written bits so start=True is only needed once.
                xa = xa_ps.tile([128, GRP], f32, tag="xa", name=f"xa{g}")
                for h in range(sub):
                    b = g * sub + h
                    for k in range(KD):
                        nc.tensor.matmul(
                            xa[:, h * blk : (h + 1) * blk],
                            lhsT=a_sl(b, k),
                            rhs=x_sl(g, k, h * blk, (h + 1) * blk),
                            start=(h == 0 and k == 0),
                            stop=(h == sub - 1 and k == KD - 1),
                            skip_group_check=True,
                        )
                t = xm_pool.tile([128, GRP], bf, tag="xm", name=f"xm{g}")
                nc.vector.tensor_mul(t[:], xa[:], m_sl(g))
                xm[g] = t

            def emit_base(g, j, o_p, ks):
                for k in ks:
                    nc.tensor.matmul(
                        o_p[:],
                        lhsT=w_sl(k, j),
                        rhs=x_sl(g, k, 0, GRP),
                        start=(k == 0),
                        stop=False,
                        skip_group_check=True,
                    )

            def finish(g, j, o_p):
                for h in range(sub):
                    nc.tensor.matmul(
                        o_p[:, h * blk : (h + 1) * blk],
                        lhsT=b_sl(g, h, j),
                        rhs=xm[g][:, h * blk : (h + 1) * blk],
                        start=False,
                        stop=(h == sub - 1),
                        skip_group_check=True,
                    )
                st = st_pool.tile([128, GRP], bf, tag="st", name=f"st{g}_{j}")
                # bias-add on the ACT engine: keeps the DVE free for the mask
                # muls so no cross-engine wait ever blocks the DVE FIFO head
                nc.scalar.add(st[:], o_p[:], bias_t[:, j : j + 1])
                # stores issue on sync: ACT-issued DMAs wait on sem-lane
                # recycling gates and would block later adds in the ACT FIFO
                nc.sync.dma_start(out_d[j, :, g * GRP : (g + 1) * GRP], st[:])

            # --- group 0: wave phase, 7 persistent banks in k order ---
            o7 = [
                out_ps.tile([128, GRP], f32, tag="o", name=f"o0_{j}")
                for j in range(7)
            ]
            for k in range(KD):
                # hold two of the last wave's matmuls back: they fill the
                # PE bubble while the mask-mul DVE round-trip completes
                nj = 5 if k == KD - 1 else 7
                for j in range(nj):
                    nc.tensor.matmul(
                        o7[j][:],
                        lhsT=w_sl(k, j),
                        rhs=wv_t[k][:, 0:GRP],
                        start=(k == 0),
                        stop=False,
                        skip_group_check=True,
                    )
            emit_xa(0)
            for j in (5, 6):
                nc.tensor.matmul(
                    o7[j][:],
                    lhsT=w_sl(KD - 1, j),
                    rhs=wv_t[KD - 1][:, 0:GRP],
                    start=False,
                    stop=False,
                    skip_group_check=True,
                )
            for j in range(7):
                finish(0, j, o7[j])
                if j == 2:
                    emit_xa(1)
            o_p = out_ps.tile([128, GRP], f32, tag="o", name="o0_7")
            emit_base(0, 7, o_p, range(KD))
            finish(0, 7, o_p)

            # --- groups 1..3: straight pipeline; final j split in half so
            # the last DVE+store chain overlaps the closing matmuls ---
            for g in range(1, NG):
                for j in range(KD):
                    last = g == NG - 1 and j == KD - 1
                    if not last:
                        o_p = out_ps.tile(
                            [128, GRP], f32, tag="o", name=f"o{g}_{j}"
                        )
                        emit_base(g, j, o_p, range(KD))
                        finish(g, j, o_p)
                        if j == 3 and g < NG - 1:
                            emit_xa(g + 1)
                        continue
                    # split pieces: each gets its own PSUM bank so earlier
                    # pieces' bias+store run while later matmuls still
                    # stream (PE-W + DVE-R same bank is fatal); the final
                    # piece is half-sized to shorten the very last
                    # add+store chain after the last matmul
                    cuts = [h * blk for h in range(sub)]
                    cuts += [(sub - 1) * blk + blk // 2, GRP]
                    pieces = list(zip(cuts[:-1], cuts[1:]))
                    for pi, (c0, c1) in enumerate(pieces):
                        h = c0 // blk
                        o_h = out_ps.tile(
                            [128, GRP], f32, tag="o", name=f"o{g}_{j}_p{pi}"
                        )
                        for k in range(KD):
                            nc.tensor.matmul(
                                o_h[:, 0 : c1 - c0],
                                lhsT=w_sl(k, j),
                                rhs=x_sl(g, k, c0, c1),
                                start=(k == 0),
                                stop=False,
                                skip_group_check=True,
                            )
                        nc.tensor.matmul(
                            o_h[:, 0 : c1 - c0],
                            lhsT=b_sl(g, h, j),
                            rhs=xm[g][:, c0:c1],
                            start=False,
                            stop=True,
                            skip_group_check=True,
                        )
                        st = st_pool.tile(
                            [128, c1 - c0], bf, tag=f"stp{pi}", name=f"st{g}_{j}_p{pi}"
                        )
                        nc.scalar.add(st[:], o_h[:, 0 : c1 - c0], bias_t[:, j : j + 1])
                        nc.sync.dma_start(
                            out_d[j, :, g * GRP + c0 : g * GRP + c1], st[:]
                        )

    nc.compile()
    return nc


def _pick_blocks(labels: np.ndarray):
    # smallest block count whose static expert windows cover every core's
    # sorted blocks (sorted blocks span a contiguous expert range)
    for n_blocks in (8, 16, 32):
        blk = TPC // n_blocks
        elo = _elo_table(n_blocks)
        ok = True
        for c in range(N_CORES):
            sl = np.sort(labels[c * TPC : (c + 1) * TPC])
            for b in range(n_blocks):
                seg = sl[b * blk : (b + 1) * blk]
                if seg[0] < elo[b] or seg[-1] >= elo[b] + SLOTS:
                    ok = False
                    break
            if not ok:
                break
        if ok:
            return n_blocks, elo
    raise ValueError("no block size with a static 16-expert window fits")


def kernel(x, labels, W, A, B, bias):
    global _last_in_maps
    x = np.asarray(x, dtype=np.float32)
    labels_i = np.asarray(labels).astype(np.int64)
    W = np.asarray(W, dtype=np.float32)
    A = np.asarray(A, dtype=np.float32)
    B = np.asarray(B, dtype=np.float32)
    bias = np.asarray(bias, dtype=np.float32)

    n_blocks, elo = _pick_blocks(labels_i)
    blk = TPC // n_blocks
    sub = GRP // blk

    if n_blocks not in _compiled:
        _compiled[n_blocks] = _build_nc(n_blocks)
    nc = _compiled[n_blocks]

    w_part = W.reshape(KD, 128, D).astype(BF16)             # [k, p, j*128+..]
    bias_in = np.ascontiguousarray(bias.reshape(KD, 128).T)  # [128, KD] f32
    B_scaled = B * SCALING
    # shared A table: atab[p, k*E*R + e*R + r] = A[e, 128k+p, r]
    a_tab = np.ascontiguousarray(
        A.astype(BF16).reshape(E, KD, 128, R).transpose(2, 1, 0, 3).reshape(
            128, KD * E * R
        )
    )

    in_maps = []
    perms = []
    for c in range(N_CORES):
        lc = labels_i[c * TPC : (c + 1) * TPC]
        perm = np.argsort(lc, kind="stable")
        perms.append(perm)
        ls = lc[perm]                          # sorted labels
        xs = x[c * TPC : (c + 1) * TPC][perm]  # [TPC, D] sorted tokens

        # xt[k, p, g, t] = xs[g*GRP + t, 128k + p]
        xt = xs.astype(BF16).T.reshape(KD, 128, NG, GRP)
        wv_in = np.ascontiguousarray(
            np.concatenate([xt[:, :, 0, :], w_part], axis=2)  # [KD, 128, WVC]
        )
        xr_in = np.ascontiguousarray(
            xt[:, :, 1:, :].transpose(2, 1, 0, 3).reshape(NG - 1, 128, KD * GRP)
        )

        in_map = {"wv": wv_in, "atab": a_tab, "bias": bias_in, "xr": xr_in}
        for g in range(NG):
            ltg = np.zeros((128, sub * D + GRP), dtype=BF16)
            for h in range(sub):
                b = g * sub + h
                seg = ls[b * blk : (b + 1) * blk]
                lo = elo[b]
                for e in np.unique(seg):
                    i = int(e) - lo
                    assert 0 <= i < SLOTS
                    ltg[i * R : (i + 1) * R, h * D : (h + 1) * D] = B_scaled[e]
                    ltg[
                        i * R : (i + 1) * R,
                        sub * D + h * blk : sub * D + (h + 1) * blk,
                    ] = (seg == e)[None, :]
            in_map[f"lt{g}"] = ltg
        in_maps.append(in_map)

    _last_in_maps = in_maps
    res = run_bass_kernel_spmd(nc, in_maps, core_ids=list(range(N_CORES)))

    out = np.empty((T, D), dtype=np.float32)
    for c in range(N_CORES):
        o_t = res.results[c]["outT"].reshape(D, TPC)  # [d, t] sorted, bf16
        out[c * TPC + perms[c]] = o_t.T.astype(np.float32)
    return out


# revision 22
# speedup vs baseline: 1.0645x; 1.0131x over previous
"""MoE-LoRA linear layer (T=16384, D=1024, E=64, R=8) on 8 Trainium2 cores.

Strategy: data-parallel over tokens (2048 tokens/core). Inside each core
everything is computed transposed (d on partitions, tokens on the free dim)
so every matmul consumes operands in their natural layout with no on-device
transposes:

  out_T[:, g] = sum_k W_k^T @ xT_k[:, g]      base GEMM, N=512 token groups
  out_T[:, b] += B_blk^T @ (mask_b * (A_blk^T @ xT[:, b]))   rank-8 LoRA

Routing is resolved on the host: each core's tokens are sorted by expert
label and cut into 256-token blocks. Because sorted blocks span a small
contiguous expert range, each block reads its LoRA A columns from ONE shared
A-table in SBUF through a compile-time static 16-expert window (elo[b]),
so A is DMA'd once (1 MB) instead of per-block packed copies. B and the
per-token masks are still host-packed per block (slot = e - elo[b]).

The start is latency-optimized: eight small "waves" [x0 chunk k | W chunk k]
(384 KB each) stream in k order, so the first matmul fires after one wave
(~1 MB earlier than a k-pair layout) and the PE never outruns the 16-engine
DMA stream (~420 GB/s aggregate). Group 0 accumulates 6 output j-chunks in
6 PSUM banks during the wave phase (the other 2 banks hold the packed
[128,512] xa pair tiles); j=6,7 run right after from resident tiles. A few
throwaway warm-up matmuls bridge the fixed ~7.5us framework preamble so the
PE clock gate (HAM, 1.2 -> 2.4 GHz) releases early. Output is stored bf16
(halves store traffic; host casts back to f32). Compute in bf16 (f32 PSUM):
fp32 matmul on TRN2 runs at 1/4 rate and would be hopelessly PE-bound.
"""

import numpy as np
import ml_dtypes

import concourse.bacc as bacc
import concourse.mybir as mybir
from concourse import tile
from concourse.bass_utils import run_bass_kernel_spmd

T, D, E, R = 16384, 1024, 64, 8
N_CORES = 8
TPC = T // N_CORES          # tokens per core
KD = D // 128               # 8 contraction chunks
GRP = 512                   # base-GEMM token group (one PSUM bank)
NG = TPC // GRP             # 4 groups
SCALING = 1.0 / R
SLOTS = 128 // R            # experts a 128-row slot window holds
N_WARM = 7                  # HAM warm-up matmuls bridging the DMA lead-in
WVC = GRP + D               # wave row: x0 chunk | W chunk

BF16 = ml_dtypes.bfloat16

_compiled = {}              # n_blocks -> Bacc program (reused across calls)
_last_in_maps = None


def _elo_table(n_blocks: int):
    # static expert window base per block: block b of sorted tokens spans
    # experts around epb*b; center a SLOTS-wide window there.
    epb = E // n_blocks
    return [
        min(max(epb * b - (SLOTS - epb) // 2, 0), E - SLOTS)
        for b in range(n_blocks)
    ]


def _build_nc(n_blocks: int):
    blk = TPC // n_blocks   # lora block (256 default)
    sub = GRP // blk        # lora blocks per token group
    elo = _elo_table(n_blocks)
    bf = mybir.dt.bfloat16
    f32 = mybir.dt.float32

    nc = bacc.Bacc(
        "TRN2", target_bir_lowering=False, debug=False, num_devices=N_CORES
    )
    wv_d = nc.dram_tensor("wv", [KD, 128, WVC], bf, kind="ExternalInput")
    a_d = nc.dram_tensor("atab", [128, KD * E * R], bf, kind="ExternalInput")
    lt_d = [
        nc.dram_tensor(f"lt{g}", [128, sub * D + GRP], bf, kind="ExternalInput")
        for g in range(NG)
    ]
    xr_d = nc.dram_tensor("xr", [NG - 1, 128, KD * GRP], bf, kind="ExternalInput")
    bias_d = nc.dram_tensor("bias", [128, KD], f32, kind="ExternalInput")
    out_d = nc.dram_tensor("outT", [KD, 128, TPC], bf, kind="ExternalOutput")

    with tile.TileContext(nc) as tc:
        with (
            tc.tile_pool(name="consts", bufs=1) as cpool,
            tc.tile_pool(name="xa_ps", bufs=1, space="PSUM") as xa_ps,
            tc.tile_pool(name="out_ps", bufs=7, space="PSUM") as out_ps,
            tc.tile_pool(name="xm_sb", bufs=2) as xm_pool,
            tc.tile_pool(name="st_sb", bufs=12) as st_pool,
        ):
            wv_t = [
                cpool.tile([128, WVC], bf, tag=f"wv{k}", name=f"wv_t{k}")
                for k in range(KD)
            ]
            a_t = cpool.tile([128, KD * E * R], bf, tag="atab", name="a_t")
            lt_t = [
                cpool.tile([128, sub * D + GRP], bf, tag=f"lt{g}", name=f"lt_t{g}")
                for g in range(NG)
            ]
            xr_t = [
                cpool.tile([128, KD * GRP], bf, tag=f"xr{g}", name=f"xr_t{g}")
                for g in range(NG - 1)
            ]
            bias_t = cpool.tile([128, KD], f32, tag="bias", name="bias_t")
            warm_sb = cpool.tile([128, GRP], bf, tag="warm", name="warm_sb")

            # issue order == arrival order on the sync HWDGE ring; every
            # tensor is ordered by first use (ACT-ring experiments measured
            # neutral-to-worse: sem-lane gates + ring competition)
            for k in range(KD):
                nc.sync.dma_start(wv_t[k][:], wv_d[k, :, :])
            nc.sync.dma_start(a_t[:], a_d[:, :])
            nc.sync.dma_start(lt_t[0][:], lt_d[0][:, :])
            nc.sync.dma_start(bias_t[:], bias_d[:, :])
            nc.sync.dma_start(xr_t[0][:], xr_d[0, :, :])
            nc.sync.dma_start(lt_t[1][:], lt_d[1][:, :])
            nc.sync.dma_start(xr_t[1][:], xr_d[1, :, :])
            nc.sync.dma_start(lt_t[2][:], lt_d[2][:, :])
            nc.sync.dma_start(xr_t[2][:], xr_d[2, :, :])
            nc.sync.dma_start(lt_t[3][:], lt_d[3][:, :])

            def w_sl(k, j):
                return wv_t[k][:, GRP + j * 128 : GRP + (j + 1) * 128]

            def x_sl(g, k, c0, c1):
                if g == 0:
                    return wv_t[k][:, c0:c1]
                return xr_t[g - 1][:, k * GRP + c0 : k * GRP + c1]

            def a_sl(b, k):
                o = k * E * R + elo[b] * R
                return a_t[:, o : o + 128]

            def b_sl(g, h, j):
                return lt_t[g][:, h * D + j * 128 : h * D + (j + 1) * 128]

            def m_sl(g):
                return lt_t[g][:, sub * D : sub * D + GRP]

            # PE warm-up across the fixed framework preamble / first-wave DMA
            nc.vector.memset(warm_sb[:], 0.0)
            for i in range(N_WARM):
                wp = out_ps.tile([128, GRP], f32, tag="o", name=f"warm_ps{i}")
                nc.tensor.matmul(
                    wp[:],
                    lhsT=warm_sb[:, 0:128],
                    rhs=warm_sb[:],
                    start=True,
                    stop=True,
                    skip_group_check=True,
                )

            xm = [None] * NG

            def emit_xa(g):
                # xa[slot, t] for both lora blocks of group g, packed into
                # one PSUM bank; each block's first write lands on cleared
                # has_written bits so start=True is only needed once.
                xa = xa_ps.tile([128, GRP], f32, tag="xa", name=f"xa{g}")
                for h in range(sub):
                    b = g * sub + h
                    for k in range(KD):
                        nc.tensor.matmul(
                            xa[:, h * blk : (h + 1) * blk],
                            lhsT=a_sl(b, k),
                            rhs=x_sl(g, k, h * blk, (h + 1) * blk),
                            start=(h == 0 and k == 0),
                            stop=(h == sub - 1 and k == KD - 1),
                            skip_group_check=True,
                        )
                t = xm_pool.tile([128, GRP], bf, tag="xm", name=f"xm{g}")
                nc.vector.tensor_mul(t[:], xa[:], m_sl(g))
                xm[g] = t

            def emit_base(g, j, o_p, ks):
                for k in ks:
                    nc.tensor.matmul(
                        o_p[:],
                        lhsT=w_sl(k, j),
                        rhs=x_sl(g, k, 0, GRP),
                        start=(k == 0),
                        stop=False,
                        skip_group_check=True,
                    )

            def finish(g, j, o_p):
                for h in range(sub):
                    nc.tensor.matmul(
                        o_p[:, h * blk : (h + 1) * blk],
                        lhsT=b_sl(g, h, j),
                        rhs=xm[g][:, h * blk : (h + 1) * blk],
                        start=False,
                        stop=(h == sub - 1),
                        skip_group_check=True,
                    )
                st = st_pool.tile([128, GRP], bf, tag="st", name=f"st{g}_{j}")
                # bias-add on the ACT engine: keeps the DVE free for the mask
                # muls so no cross-engine wait ever blocks the DVE FIFO head
                nc.scalar.add(st[:], o_p[:], bias_t[:, j : j + 1])
                # stores issue on sync: ACT-issued DMAs wait on sem-lane
                # recycling gates and would block later adds in the ACT FIFO
                nc.sync.dma_start(out_d[j, :, g * GRP : (g + 1) * GRP], st[:])

            # --- group 0: wave phase, 7 persistent banks in k order ---
            o7 = [
                out_ps.tile([128, GRP], f32, tag="o", name=f"o0_{j}")
                for j in range(7)
            ]
            for k in range(KD):
                # hold four of the last wave's matmuls back: they fill the
                # PE bubble while the mask-mul DVE round-trip completes
                nj = 3 if k == KD - 1 else 7
                for j in range(nj):
                    nc.tensor.matmul(
                        o7[j][:],
                        lhsT=w_sl(k, j),
                        rhs=wv_t[k][:, 0:GRP],
                        start=(k == 0),
                        stop=False,
                        skip_group_check=True,
                    )
            emit_xa(0)
            for j in (3, 4, 5, 6):
                nc.tensor.matmul(
                    o7[j][:],
                    lhsT=w_sl(KD - 1, j),
                    rhs=wv_t[KD - 1][:, 0:GRP],
                    start=False,
                    stop=False,
                    skip_group_check=True,
                )
            for j in range(7):
                finish(0, j, o7[j])
                if j == 2:
                    emit_xa(1)
            o_p = out_ps.tile([128, GRP], f32, tag="o", name="o0_7")
            emit_base(0, 7, o_p, range(KD))
            finish(0, 7, o_p)

            # --- groups 1..3: straight pipeline; final j split in half so
            # the last DVE+store chain overlaps the closing matmuls ---
            for g in range(1, NG):
                for j in range(KD):
                    last = g == NG - 1 and j == KD - 1
                    if not last:
                        o_p = out_ps.tile(
                            [128, GRP], f32, tag="o", name=f"o{g}_{j}"
                        )
                        emit_base(g, j, o_p, range(KD))
                        finish(g, j, o_p)
                        if j == 3 and g < NG - 1:
                            emit_xa(g + 1)
                        continue
                    # split pieces: each gets its own PSUM bank so earlier
                    # pieces' bias+store run while later matmuls still
                    # stream (PE-W + DVE-R same bank is fatal); the final
                    # piece is half-sized to shorten the very last
                    # add+store chain after the last matmul
                    cuts = [h * blk for h in range(sub)]
                    cuts += [(sub - 1) * blk + blk // 2, GRP]
                    pieces = list(zip(cuts[:-1], cuts[1:]))
                    for pi, (c0, c1) in enumerate(pieces):
                        h = c0 // blk
                        o_h = out_ps.tile(
                            [128, GRP], f32, tag="o", name=f"o{g}_{j}_p{pi}"
                        )
                        for k in range(KD):
                            nc.tensor.matmul(
                                o_h[:, 0 : c1 - c0],
                                lhsT=w_sl(k, j),
                                rhs=x_sl(g, k, c0, c1),
                                start=(k == 0),
                                stop=False,
                                skip_group_check=True,
                            )
                        nc.tensor.matmul(
                            o_h[:, 0 : c1 - c0],
                            lhsT=b_sl(g, h, j),
                            rhs=xm[g][:, c0:c1],
                            start=False,
                            stop=True,
                            skip_group_check=True,
                        )
                        st = st_pool.tile(
                            [128, c1 - c0], bf, tag=f"stp{pi}", name=f"st{g}_{j}_p{pi}"
                        )
                        nc.scalar.add(st[:], o_h[:, 0 : c1 - c0], bias_t[:, j : j + 1])
                        nc.sync.dma_start(
                            out_d[j, :, g * GRP + c0 : g * GRP + c1], st[:]
                        )

    nc.compile()
    return nc


def _pick_blocks(labels: np.ndarray):
    # smallest block count whose static expert windows cover every core's
    # sorted blocks (sorted blocks span a contiguous expert range)
    for n_blocks in (8, 16, 32):
        blk = TPC // n_blocks
        elo = _elo_table(n_blocks)
        ok = True
        for c in range(N_CORES):
            sl = np.sort(labels[c * TPC : (c + 1) * TPC])
            for b in range(n_blocks):
                seg = sl[b * blk : (b + 1) * blk]
                if seg[0] < elo[b] or seg[-1] >= elo[b] + SLOTS:
                    ok = False
                    break
            if not ok:
                break
        if ok:
            return n_blocks, elo
    raise ValueError("no block size with a static 16-expert window fits")


def kernel(x, labels, W, A, B, bias):
    global _last_in_maps
    x = np.asarray(x, dtype=np.float32)
    labels_i = np.asarray(labels).astype(np.int64)
    W = np.asarray(W, dtype=np.float32)
    A = np.asarray(A, dtype=np.float32)
    B = np.asarray(B, dtype=np.float32)
    bias = np.asarray(bias, dtype=np.float32)

    n_blocks, elo = _pick_blocks(labels_i)
    blk = TPC // n_blocks
    sub = GRP // blk

    if n_blocks not in _compiled:
        _compiled[n_blocks] = _build_nc(n_blocks)
    nc = _compiled[n_blocks]

    w_part = W.reshape(KD, 128, D).astype(BF16)             # [k, p, j*128+..]
    bias_in = np.ascontiguousarray(bias.reshape(KD, 128).T)  # [128, KD] f32
    B_scaled = B * SCALING
    # shared A table: atab[p, k*E*R + e*R + r] = A[e, 128k+p, r]
    a_tab = np.ascontiguousarray(
        A.astype(BF16).reshape(E, KD, 128, R).transpose(2, 1, 0, 3).reshape(
            128, KD * E * R
        )
    )

    in_maps = []
    perms = []
    for c in range(N_CORES):
        lc = labels_i[c * TPC : (c + 1) * TPC]
        perm = np.argsort(lc, kind="stable")
        perms.append(perm)
        ls = lc[perm]                          # sorted labels
        xs = x[c * TPC : (c + 1) * TPC][perm]  # [TPC, D] sorted tokens

        # xt[k, p, g, t] = xs[g*GRP + t, 128k + p]
        xt = xs.astype(BF16).T.reshape(KD, 128, NG, GRP)
        wv_in = np.ascontiguousarray(
            np.concatenate([xt[:, :, 0, :], w_part], axis=2)  # [KD, 128, WVC]
        )
        xr_in = np.ascontiguousarray(
            xt[:, :, 1:, :].transpose(2, 1, 0, 3).reshape(NG - 1, 128, KD * GRP)
        )

        in_map = {"wv": wv_in, "atab": a_tab, "bias": bias_in, "xr": xr_in}
        for g in range(NG):
            ltg = np.zeros((128, sub * D + GRP), dtype=BF16)
            for h in range(sub):
                b = g * sub + h
                seg = ls[b * blk : (b + 1) * blk]
                lo = elo[b]
                for e in np.unique(seg):
                    i = int(e) - lo
                    assert 0 <= i < SLOTS
                    ltg[i * R : (i + 1) * R, h * D : (h + 1) * D] = B_scaled[e]
                    ltg[
                        i * R : (i + 1) * R,
                        sub * D + h * blk : sub * D + (h + 1) * blk,
                    ] = (seg == e)[None, :]
            in_map[f"lt{g}"] = ltg
        in_maps.append(in_map)

    _last_in_maps = in_maps
    res = run_bass_kernel_spmd(nc, in_maps, core_ids=list(range(N_CORES)))

    out = np.empty((T, D), dtype=np.float32)
    for c in range(N_CORES):
        o_t = res.results[c]["outT"].reshape(D, TPC)  # [d, t] sorted, bf16
        out[c * TPC + perms[c]] = o_t.T.astype(np.float32)
    return out
